# revision 1
# baseline (speedup 1.0000x reference)
"""Distributed causal self-attention kernel for one TRN2 chip (8 NeuronCores).

Problem: y = CausalSelfAttention(x) with B=2, T=2048, C=1024, 16 heads x 64.

Sharding (per core c = b*4 + hg;  b = batch, hg = head-group of 4 heads):
  - Q/K/V projections: column-sharded per head group (each core computes its
    4 heads' Q,K,V from the full x of its batch).
  - Attention: fully local (4 heads per core), flash-style, scores kept
    transposed (s^T[k, q]) so no on-chip transposes are needed.
  - Row-sums for softmax ride the AV matmul as a 65th "ones" column of V.
  - y^T shards are AllGathered within each batch group of 4 cores (two
    gathers, one per head-pair, so comm overlaps the second pair's compute).
  - o_proj: each core computes its own 256 output columns from the full
    gathered y^T -> output shards are disjoint; the host just concatenates.

All matmuls run in bf16 (fp32 accumulation in PSUM); inputs are converted to
bf16 on the host. QK^T matmuls (contraction dim 64) are packed two-per-PE
via tile_position row tiling.
"""
import sys
sys.path.insert(0, '/opt/trn_rl_repo')
import numpy as np
import ml_dtypes

B, T, C = 2, 2048, 1024
NH, HD = 16, 64
N_CORES = 8
GROUPS = [[0, 1, 2, 3], [4, 5, 6, 7]]
HPC = NH // 4            # heads per core = 4
SH = HPC * HD            # per-core projection width = 256
NCB = C // 128           # contraction blocks = 8
QT = 512                 # query tile
BF16 = ml_dtypes.bfloat16

_CACHE = {}


def _build(t_len):
    import concourse.bass as bass
    import concourse.bacc as bacc
    import concourse.tile as tile
    import concourse.mybir as mybir
    dt = mybir.dt
    f32, bf16 = dt.float32, dt.bfloat16

    nqt = t_len // QT        # query tiles
    ntc = t_len // 128       # t chunks of 128
    VW = HPC * 65            # vhat row width = 260

    nc = bacc.Bacc("TRN2", target_bir_lowering=False, debug=False,
                   num_devices=N_CORES)
    # inputs arrive pre-blocked on the host: [(cblk p) ...] -> [p, cblk*...]
    xT = nc.dram_tensor("xT", [128, NCB * t_len], bf16, kind="ExternalInput")
    wq = nc.dram_tensor("wqT", [128, NCB * SH], bf16, kind="ExternalInput")
    wk = nc.dram_tensor("wkT", [128, NCB * SH], bf16, kind="ExternalInput")
    wv = nc.dram_tensor("wvT", [128, NCB * SH], bf16, kind="ExternalInput")
    wo = nc.dram_tensor("woT", [128, NCB * SH], bf16, kind="ExternalInput")
    masks = nc.dram_tensor("masks", [128, 1024], bf16, kind="ExternalInput")
    out = nc.dram_tensor("out", [SH, t_len], bf16, kind="ExternalOutput")

    with tile.TileContext(nc) as tc:
        with tc.tile_pool(name="big", bufs=1) as big, \
             tc.tile_pool(name="epool", bufs=6) as epool, \
             tc.tile_pool(name="small", bufs=3) as small, \
             tc.tile_pool(name="ygp", bufs=16) as ygp, \
             tc.tile_pool(name="stp", bufs=3) as stp, \
             tc.tile_pool(name="ps", bufs=4, space="PSUM") as psp, \
             tc.tile_pool(name="dram", bufs=1, space="DRAM") as dram:

            # ---- resident SBUF tensors ----
            xt = big.tile([128, NCB * t_len], bf16)       # x^T, c-blocked
            wq_sb = big.tile([128, NCB * SH], bf16)
            wk_sb = big.tile([128, NCB * SH], bf16)
            wv_sb = big.tile([128, NCB * SH], bf16)
            wo_sb = big.tile([128, NCB * SH], bf16)
            mask_sb = big.tile([128, 1024], bf16)
            qt_sb = big.tile([128, 2 * t_len], bf16)      # Q^T, pair-blocked
            kt_sb = big.tile([128, 2 * t_len], bf16)
            vhat_sb = big.tile([128, ntc * VW], bf16)     # [V_h | 1] per head

            # DMA emission in first-consumer order: the first Q-proj matmul
            # needs wq[0]+xt[0]; interleave so PE starts ~1.5us in, not ~12us.
            for k in range(NCB):
                nc.sync.dma_start(wq_sb[:, k * SH:(k + 1) * SH],
                                  wq[:, k * SH:(k + 1) * SH])
                for hh in (0, 1):
                    nc.sync.dma_start(
                        xt[:, k * t_len + hh * (t_len // 2): k * t_len + (hh + 1) * (t_len // 2)],
                        xT[:, k * t_len + hh * (t_len // 2): k * t_len + (hh + 1) * (t_len // 2)])
            for w_sb, w_in in ((wk_sb, wk), (wv_sb, wv), (wo_sb, wo)):
                for k in range(NCB):
                    nc.sync.dma_start(w_sb[:, k * SH:(k + 1) * SH],
                                      w_in[:, k * SH:(k + 1) * SH])
            nc.sync.dma_start(mask_sb[:], masks[:])
            nc.gpsimd.memset(vhat_sb[:], 1.0)

            # ---- DRAM bounce buffers for the AllGathers (pair x t-half) ----
            n_th = max(1, t_len // 1024)
            th_len = t_len // n_th
            agin = [[dram.tile([128, th_len], bf16, name=f"agin{p}{th}")
                     for th in range(n_th)] for p in (0, 1)]
            agout = [[dram.tile([512, th_len], bf16, name=f"agout{p}{th}")
                      for th in range(n_th)] for p in (0, 1)]

            def qk_proj(pair, w_sb, dst_sb):
                """Q^T/K^T for one head pair: dst rows = head dims (2x64)."""
                for n in range(nqt):
                    ps = psp.tile([128, 512], f32, name="ps")
                    for k in range(NCB):
                        nc.tensor.matmul(
                            ps[:],
                            lhsT=w_sb[:, k * SH + pair * 128: k * SH + (pair + 1) * 128],
                            rhs=xt[:, k * t_len + n * QT: k * t_len + n * QT + QT],
                            start=(k == 0), stop=(k == NCB - 1))
                    nc.vector.tensor_copy(
                        dst_sb[:, pair * t_len + n * QT: pair * t_len + n * QT + QT],
                        ps[:])

            def v_proj():
                """V in [t, o] layout, written per head into vhat (col 65 stays 1)."""
                for tch in range(ntc):
                    ps = psp.tile([128, SH], f32, name="ps")
                    for k in range(NCB):
                        nc.tensor.matmul(
                            ps[:],
                            lhsT=xt[:, k * t_len + tch * 128: k * t_len + (tch + 1) * 128],
                            rhs=wv_sb[:, k * SH:(k + 1) * SH],
                            start=(k == 0), stop=(k == NCB - 1))
                    for h in range(HPC):
                        nc.vector.tensor_copy(
                            vhat_sb[:, tch * VW + h * 65: tch * VW + h * 65 + 64],
                            ps[:, h * 64:(h + 1) * 64])

            def attention(pair):
                def qk_mm(dst, kb, qa, w, h01):
                    """s^T block matmul: k-block kb vs q cols [qa, qa+w)."""
                    nc.tensor.matmul(
                        dst,
                        lhsT=kt_sb[h01 * 64:(h01 + 1) * 64,
                                   pair * t_len + kb * 128: pair * t_len + (kb + 1) * 128],
                        rhs=qt_sb[h01 * 64:(h01 + 1) * 64,
                                  pair * t_len + qa: pair * t_len + qa + w],
                        start=True, stop=True,
                        tile_position=(h01 * 64, 0))

                def av_mm(aug, e_slice, kb, h01, ca, w, start, stop):
                    h = pair * 2 + h01
                    return nc.tensor.matmul(
                        aug[0:65, h01 * 512 + ca: h01 * 512 + ca + w],
                        lhsT=vhat_sb[:, kb * VW + h * 65: kb * VW + (h + 1) * 65],
                        rhs=e_slice,
                        start=start, stop=stop,
                        skip_group_check=True)

                for qi in range(nqt):
                    q0 = qi * QT
                    nfull = q0 // 128          # k-blocks fully valid for all 512 q
                    aug = psp.tile([128, 1024], f32, name="ps")
                    for kb in range(nfull):
                        qk = psp.tile([128, 1024], f32, name="ps")
                        for h01 in (0, 1):
                            qk_mm(qk[:, h01 * 512:(h01 + 1) * 512], kb, q0, 512, h01)
                        e = epool.tile([128, 1024], bf16, name="e")
                        nc.scalar.activation(e[:], qk[:],
                                             mybir.ActivationFunctionType.Exp,
                                             scale=1.0 / np.sqrt(HD))
                        for h01 in (0, 1):
                            av_mm(aug, e[:, h01 * 512:(h01 + 1) * 512], kb, h01,
                                  0, 512, start=(kb == 0), stop=False)
                    # mid supertile: blocks nfull, nfull+1 are fully valid for
                    # the upper q-half [q0+256, q0+512). Packed (i, h01) x 256.
                    mid = psp.tile([128, 1024], f32, name="ps")
                    for i in (0, 1):
                        for h01 in (0, 1):
                            qk_mm(mid[:, (h01 * 2 + i) * 256:(h01 * 2 + i + 1) * 256],
                                  nfull + i, q0 + 256, 256, h01)
                    em = epool.tile([128, 1024], bf16, name="e")
                    nc.scalar.activation(em[:], mid[:],
                                         mybir.ActivationFunctionType.Exp,
                                         scale=1.0 / np.sqrt(HD))
                    for i in (0, 1):
                        for h01 in (0, 1):
                            av_mm(aug, em[:, (h01 * 2 + i) * 256:(h01 * 2 + i + 1) * 256],
                                  nfull + i, h01, 256, 256,
                                  start=(nfull == 0 and i == 0), stop=False)
                    # Two diagonal bands: band u covers q-half [q0+u*256, +256)
                    # against k-blocks nfull+2u, nfull+2u+1 with the causal mask.
                    # PSUM accumulation-group discipline: a start=True matmul
                    # into a bank clobbers any OPEN group in that bank, so band1
                    # (which closes the [256,512) group opened by full/mid) must
                    # fully precede band0's start when nfull==0. Band order
                    # (1, 0) plus an explicit dep enforces this.
                    band_last_av = None
                    band0_first_av = None
                    for u in (1, 0):
                        bd = psp.tile([128, 1024], f32, name="ps")
                        for i in (0, 1):
                            for h01 in (0, 1):
                                qk_mm(bd[:, (h01 * 2 + i) * 256:(h01 * 2 + i + 1) * 256],
                                      nfull + 2 * u + i, q0 + u * 256, 256, h01)
                        eb = epool.tile([128, 1024], bf16, name="e")
                        nc.scalar.activation(eb[:], bd[:],
                                             mybir.ActivationFunctionType.Exp,
                                             scale=1.0 / np.sqrt(HD))
                        nc.vector.tensor_mul(eb[:], eb[:], mask_sb[:])
                        for i in (0, 1):
                            for h01 in (0, 1):
                                av = av_mm(aug, eb[:, (h01 * 2 + i) * 256:(h01 * 2 + i + 1) * 256],
                                           nfull + 2 * u + i, h01, u * 256, 256,
                                           start=(nfull == 0 and u == 0 and i == 0),
                                           stop=(i == 1))
                                if u == 1:
                                    band_last_av = av
                                elif band0_first_av is None:
                                    band0_first_av = av
                    if nfull == 0 and band_last_av is not None:
                        tile.add_dep_helper(band0_first_av.ins, band_last_av.ins,
                                            reason="bank group: band0 start after band1 closes")
                    # normalize: y^T_h = aug[0:64] / aug[64]
                    recip = small.tile([1, 1024], bf16, name="recip")
                    with nc.allow_low_precision(reason="softmax denom in bf16 is within tolerance"):
                        nc.vector.reciprocal(recip[:], aug[64:65, 0:1024])
                    bc = small.tile([64, 1024], bf16, name="bc")
                    nc.gpsimd.partition_broadcast(bc[:], recip[:])
                    yt = small.tile([64, 1024], bf16, name="yt")
                    nc.vector.tensor_mul(yt[:], aug[0:64, 0:1024], bc[:])
                    th, tq = divmod(q0, th_len)
                    nc.sync.dma_start(
                        agin[pair][th].rearrange("(h d) t -> d h t", h=2)[:, :, tq:tq + QT],
                        yt.rearrange("d (h t) -> d h t", h=2))

            # ---- schedule: pair0 projections -> attention p0 (ACT-bound)
            #      overlapped with pair1 projections -> attention p1 ----
            qk_proj(0, wq_sb, qt_sb)
            qk_proj(0, wk_sb, kt_sb)
            v_proj()
            attention(0)
            qk_proj(1, wq_sb, qt_sb)
            qk_proj(1, wk_sb, kt_sb)
            attention(1)

            for th in range(n_th):
                for p in (0, 1):
                    nc.gpsimd.collective_compute(
                        "AllGather", mybir.AluOpType.bypass,
                        replica_groups=GROUPS,
                        ins=[agin[p][th].opt()], outs=[agout[p][th].opt()])

            # ---- gathered y^T -> SBUF ----
            ygt = {}  # (global c-block, t-half) -> sbuf tile
            for th in range(n_th):
                for p in (0, 1):
                    for r in range(4):
                        t = ygp.tile([128, th_len], bf16, name="yg")
                        nc.gpsimd.dma_start(t[:], agout[p][th][r * 128:(r + 1) * 128, :])
                        ygt[(2 * r + p, th)] = t

            def yg(cb, th):
                return ygt[(cb, th)]

            # ---- o_proj: out^T[o, t] = sum_c woT[c, o] * yg[c, t] ----
            nth = th_len // QT
            for n in range(nqt):
                th, nn = divmod(n, nth)
                for m in range(2):
                    ps = psp.tile([128, 512], f32, name="ps")
                    for cb in range(NCB):
                        nc.tensor.matmul(
                            ps[:],
                            lhsT=wo_sb[:, cb * SH + m * 128: cb * SH + (m + 1) * 128],
                            rhs=yg(cb, th)[:, nn * QT: nn * QT + QT],
                            start=(cb == 0), stop=(cb == NCB - 1))
                    st = stp.tile([128, 512], bf16, name="st")
                    nc.vector.tensor_copy(st[:], ps[:])
                    nc.sync.dma_start(
                        out[m * 128:(m + 1) * 128, n * QT: n * QT + QT], st[:])

    nc.compile()
    return nc


def _masks_np():
    """Diagonal causal mask: [ki, qi] = qi >= ki, duplicated along the free
    axis for the two packed heads."""
    ki = np.arange(128)[:, None]
    qi = np.arange(128)[None, :]
    tri = (qi >= ki).astype(np.float32)
    ones = np.ones((128, 128), np.float32)
    zeros = np.zeros((128, 128), np.float32)
    lo = np.concatenate([tri, ones], axis=1)    # lower k-block of a band
    hi = np.concatenate([zeros, tri], axis=1)   # upper k-block of a band
    return np.concatenate([lo, hi, lo, hi], axis=1).astype(BF16)  # [128, 1024]


def _block(a, w):
    """[C, w] -> [128, NCB*w] partition-blocked bf16."""
    return np.ascontiguousarray(
        a.reshape(NCB, 128, w).transpose(1, 0, 2).reshape(128, NCB * w)).astype(BF16)


def _prep_inputs(x, Wq, Wk, Wv, Wo, t_len):
    masks = _masks_np()
    in_maps = []
    for c in range(N_CORES):
        b, hg = divmod(c, 4)
        sl = slice(hg * SH, (hg + 1) * SH)
        in_maps.append({
            "xT": _block(x[b].T, t_len),
            "wqT": _block(Wq[sl, :].T, SH),
            "wkT": _block(Wk[sl, :].T, SH),
            "wvT": _block(Wv[sl, :].T, SH),
            "woT": _block(Wo[sl, :].T, SH),
            "masks": masks,
        })
    return in_maps


def _assemble(results, t_len):
    out = np.empty((B, t_len, C), dtype=np.float32)
    for c in range(N_CORES):
        b, hg = divmod(c, 4)
        out[b, :, hg * SH:(hg + 1) * SH] = results[c]["out"].T.astype(np.float32)
    return out


def get_nc(t_len=T):
    if t_len not in _CACHE:
        _CACHE[t_len] = _build(t_len)
    return _CACHE[t_len]


def kernel(x, Wq, Wk, Wv, Wo):
    from concourse import bass_utils
    x = np.asarray(x, dtype=np.float32)
    nc = get_nc(T)
    in_maps = _prep_inputs(x, np.asarray(Wq), np.asarray(Wk), np.asarray(Wv),
                           np.asarray(Wo), T)
    res = bass_utils.run_bass_kernel_spmd(nc, in_maps, core_ids=list(range(N_CORES)))
    return _assemble(res.results, T)



# revision 27
# speedup vs baseline: 1.6608x; 1.6608x over previous
"""Distributed causal self-attention kernel for one TRN2 chip (8 NeuronCores).

Problem: y = CausalSelfAttention(x) with B=2, T=2048, C=1024, 16 heads x 64.

Sharding (per core c = b*4 + hg;  b = batch, hg = head-group of 4 heads):
  - Q/K/V projections: column-sharded per head group (each core computes its
    4 heads' Q,K,V from the full x of its batch).
  - Attention: fully local (4 heads per core), flash-style single pass.
    Scores are kept transposed (s^T[k, q]); AV runs "transposed" too:
    y[q, d] = e^T . [V | 1]  so each AV matmul streams only 65 columns and
    the softmax denominator lands as a per-partition (per-q) column, making
    normalization a cheap per-partition multiply.
  - y chunks are PE-transposed back to y^T[c, t] and AllGathered within each
    batch group of 4 cores in 8 t-slices of 256 so comm overlaps compute.
  - o_proj: each core computes its own 256 output columns from the full
    gathered y^T -> output shards are disjoint; the host just concatenates.

All matmuls run in bf16 (fp32 accumulation in PSUM); inputs are converted to
bf16 on the host. QK^T matmuls (contraction dim 64) are packed two-per-PE
via tile_position row tiling. Exp is sized to the exact causal area
(diagonal tiles use narrowed strided APs).
"""
import sys
sys.path.insert(0, '/opt/trn_rl_repo')
import numpy as np
import ml_dtypes

B, T, C = 2, 2048, 1024
NH, HD = 16, 64
N_CORES = 8
GROUPS = [[0, 1, 2, 3], [4, 5, 6, 7]]
HPC = NH // 4             # heads per core = 4
SH = HPC * HD             # per-core projection width = 256
NCB = C // 128            # contraction blocks = 8
VH = 68                   # padded [V_h | 1 | pad] group width
VW = HPC * VH             # vhat row width per t-chunk = 272
BF16 = ml_dtypes.bfloat16

_CACHE = {}


def _build(t_len):
    import concourse.bass as bass
    import concourse.bacc as bacc
    import concourse.tile as tile
    import concourse.mybir as mybir
    dt = mybir.dt
    f32, bf16 = dt.float32, dt.bfloat16

    ntc = t_len // 128        # t chunks of 128 (16)
    nqq = t_len // 256        # q tiles of 256 == AllGather slices (8)
    ntt = t_len // 512        # projection t tiles (4)

    nc = bacc.Bacc("TRN2", target_bir_lowering=False, debug=False,
                   num_devices=N_CORES)
    # inputs arrive pre-blocked on the host: [(cblk p) ...] -> [p, cblk*...]
    xT = nc.dram_tensor("xT", [128, NCB * t_len], bf16, kind="ExternalInput")
    wq = nc.dram_tensor("wqT", [128, NCB * SH], bf16, kind="ExternalInput")
    wk = nc.dram_tensor("wkT", [128, NCB * SH], bf16, kind="ExternalInput")
    wv = nc.dram_tensor("wvT", [128, NCB * SH], bf16, kind="ExternalInput")
    wo = nc.dram_tensor("woT", [128, NCB * SH], bf16, kind="ExternalInput")
    tri = nc.dram_tensor("tri", [128, 128], bf16, kind="ExternalInput")
    ident = nc.dram_tensor("ident", [128, 128], bf16, kind="ExternalInput")
    out = nc.dram_tensor("out", [SH, t_len], bf16, kind="ExternalOutput")

    with tile.TileContext(nc) as tc:
        with tc.tile_pool(name="big", bufs=1) as big, \
             tc.tile_pool(name="epool", bufs=20) as epool, \
             tc.tile_pool(name="ypool", bufs=3) as ypool, \
             tc.tile_pool(name="ytp", bufs=3) as ytp, \
             tc.tile_pool(name="small", bufs=4) as small, \
             tc.tile_pool(name="ygp", bufs=8) as ygp, \
             tc.tile_pool(name="stp", bufs=3) as stp, \
             tc.tile_pool(name="psqk", bufs=3, space="PSUM") as psqk, \
             tc.tile_pool(name="psaug", bufs=1, space="PSUM") as psaug, \
             tc.tile_pool(name="dram", bufs=1, space="DRAM") as dram:

            # ---- resident SBUF tensors ----
            xt = big.tile([128, NCB * t_len], bf16)       # x^T, c-blocked
            wq_sb = big.tile([128, NCB * SH], bf16)
            wk_sb = big.tile([128, NCB * SH], bf16)
            wv_sb = big.tile([128, NCB * SH], bf16)
            wo_sb = big.tile([128, NCB * SH], bf16)
            tri_sb = big.tile([128, 128], bf16)
            id_sb = big.tile([128, 128], bf16)
            qt_sb = big.tile([128, 2 * t_len], bf16)      # Q^T, pair-blocked
            kt_sb = big.tile([128, 2 * t_len], bf16)
            vhat_sb = big.tile([128, ntc * VW], bf16)     # [V_h | 1] per head

            # Coalesced input DMAs, first-consumer order.  x arrives one
            # 512-wide t tile at a time (all 8 c-blocks, strided).
            def x_tile_dma(ti):
                nc.sync.dma_start(
                    xt.rearrange("p (k t) -> p k t", k=NCB)
                    [:, :, ti * 512:(ti + 1) * 512],
                    xT.rearrange("p (k t) -> p k t", k=NCB)
                    [:, :, ti * 512:(ti + 1) * 512])

            nc.sync.dma_start(wq_sb[:, 0:SH], wq[:, 0:SH])
            for k in range(0, NCB, 2):
                nc.sync.dma_start(
                    xt.rearrange("p (k t) -> p k t", k=NCB)[:, k:k + 2, 0:512],
                    xT.rearrange("p (k t) -> p k t", k=NCB)[:, k:k + 2, 0:512])
            nc.sync.dma_start(wq_sb[:, SH:], wq[:, SH:])
            nc.sync.dma_start(wk_sb[:], wk[:])
            nc.sync.dma_start(wv_sb[:], wv[:])
            nc.sync.dma_start(tri_sb[:], tri[:])
            nc.sync.dma_start(id_sb[:], ident[:])
            for ti in range(1, ntt):
                x_tile_dma(ti)
            nc.sync.dma_start(wo_sb[:], wo[:])
            nc.gpsimd.memset(vhat_sb[:], 1.0)

            # ---- DRAM bounce buffers for the AllGathers (one per q-tile) --
            agin = [dram.tile([2 * 128, 256], bf16, name=f"agin{qq}")
                    for qq in range(nqq)]
            agout = [dram.tile([NCB * 128, 256], bf16, name=f"agout{qq}")
                     for qq in range(nqq)]

            def proj_unit(w_sb, dst, pair, t0):
                """One q/k projection psum group: [d128, t512]."""
                ps = psqk.tile([128, 512], f32, name="ps")
                for k in range(NCB):
                    nc.tensor.matmul(
                        ps[:],
                        lhsT=w_sb[:, k * SH + pair * 128:
                                  k * SH + (pair + 1) * 128],
                        rhs=xt[:, k * t_len + t0: k * t_len + t0 + 512],
                        start=(k == 0), stop=(k == NCB - 1))
                nc.vector.tensor_copy(
                    dst[:, pair * t_len + t0: pair * t_len + t0 + 512], ps[:])

            def v_unit(tch):
                """One V projection psum group: [t128, 4h x 64d] -> vhat."""
                ps = psqk.tile([128, SH], f32, name="ps")
                for k in range(NCB):
                    nc.tensor.matmul(
                        ps[:],
                        lhsT=xt[:, k * t_len + tch * 128:
                                k * t_len + (tch + 1) * 128],
                        rhs=wv_sb[:, k * SH:(k + 1) * SH],
                        start=(k == 0), stop=(k == NCB - 1))
                nc.vector.tensor_copy(
                    vhat_sb.rearrange("p (c h d) -> p c h d",
                                      c=ntc, h=HPC)[:, tch, :, 0:64],
                    ps.rearrange("p (h d) -> p h d", h=HPC))

            def proj_units(ti):
                t0 = ti * 512
                return ([(lambda p=pair, w=w_sb, d=dst: proj_unit(w, d, p, t0))
                         for w_sb, dst in ((wq_sb, qt_sb), (wk_sb, kt_sb))
                         for pair in (0, 1)] +
                        [(lambda t=tch: v_unit(t))
                         for tch in range(ti * 4, ti * 4 + 4)])

            def yg_pull(qq):
                t = ygp.tile([128, NCB * 256], bf16, name="yg")
                nc.sync.dma_start(
                    t.rearrange("p (r t) -> p r t", r=NCB),
                    agout[qq].rearrange("(r p) t -> p r t", r=NCB))
                return t

            def o_unit(qq, ygt, st2, m):
                """One o_proj output block [o128, t256] for t slice qq."""
                ps = psqk.tile([128, 256], f32, name="ps")
                for cb in range(NCB):
                    nc.tensor.matmul(
                        ps[:],
                        lhsT=wo_sb[:, cb * SH + m * 128: cb * SH + (m + 1) * 128],
                        rhs=ygt[:, cb * 256:(cb + 1) * 256],
                        start=(cb == 0), stop=(cb == NCB - 1))
                nc.vector.tensor_copy(st2[:, m * 256:(m + 1) * 256], ps[:])
                if m == 1:
                    nc.sync.dma_start(
                        out[:, qq * 256:(qq + 1) * 256]
                        .rearrange("(m p) t -> p m t", m=2),
                        st2.rearrange("p (m t) -> p m t", m=2))

            def o_units(qq, ygt):
                st2 = stp.tile([128, 512], bf16, name="st")
                return [(lambda m=m, y=ygt, s=st2: o_unit(qq, y, s, m))
                        for m in (0, 1)]

            # ---- filler: PE work units woven into ACT-bound QK stretches.
            # Entries are (deadline_qq, fn): the unit must have run before
            # attention_qtile(deadline_qq) starts (projections feeding it).
            fillq = []

            def pump(n=1):
                for _ in range(min(n, len(fillq))):
                    fillq.pop(0)[1]()

            def pump_due(qq):
                while any(d <= qq for d, _ in fillq):
                    pump(1)

            def qk_exp(qq, kb, q0):
                """QK^T + exp for one k-block; returns e tile.  PSUM column
                groups are ordered (h01, pair) so each 2KB bank only receives
                matmuls from a single PE row-tile position (bank0 <- rows
                0-63, bank1 <- rows 64-127) -- mixing positions in one bank
                does not compile.  On-diagonal blocks get the causal tri mask
                applied on DVE after the exp."""
                j = kb - 2 * qq
                qa = 128 if j == 1 else 0
                qk = psqk.tile([128, 1024], f32, name="ps")
                for pair in (0, 1):
                    for h01 in (0, 1):
                        g = h01 * 2 + pair
                        nc.tensor.matmul(
                            qk[:, g * 256 + qa: (g + 1) * 256],
                            lhsT=kt_sb[h01 * 64:(h01 + 1) * 64,
                                       pair * t_len + kb * 128:
                                       pair * t_len + (kb + 1) * 128],
                            rhs=qt_sb[h01 * 64:(h01 + 1) * 64,
                                      pair * t_len + q0 + qa:
                                      pair * t_len + q0 + 256],
                            start=True, stop=True,
                            tile_position=(h01 * 64, 0))
                e = epool.tile([128, 1024], bf16, name="e")
                if qa == 0:
                    nc.scalar.activation(e[:], qk[:],
                                         mybir.ActivationFunctionType.Exp,
                                         scale=1.0 / np.sqrt(HD))
                else:
                    e4 = e.rearrange("p (g q) -> p g q", g=HPC)
                    qk4 = qk.rearrange("p (g q) -> p g q", g=HPC)
                    nc.scalar.activation(e4[:, :, qa:256], qk4[:, :, qa:256],
                                         mybir.ActivationFunctionType.Exp,
                                         scale=1.0 / np.sqrt(HD))
                if j >= 0:
                    e4 = e.rearrange("p (g q) -> p g q", g=HPC)
                    nc.vector.tensor_mul(
                        e4[:, :, qa:qa + 128], e4[:, :, qa:qa + 128],
                        tri_sb[:].rearrange("p (o q) -> p o q", o=1)
                        .to_broadcast([128, HPC, 128]))
                return e

            def av4(aug, etile, kb, qc, start, stop):
                """AV for one (k-block, q-chunk): per head, y[q,d]+rowsum.
                qc0 lives at cols h*512+0:65 of aug, qc1 at h*512+65:130 --
                same PSUM bank, one accumulation region per head: only the
                very first write per bank carries start=True, only the very
                last carries stop=True (pending-zero covers qc1's first
                write)."""
                for h in range(HPC):
                    nc.tensor.matmul(
                        aug[:, h * 256 + qc * VH: h * 256 + qc * VH + VH],
                        lhsT=etile[:, ((h % 2) * 2 + h // 2) * 256 + qc * 128:
                                   ((h % 2) * 2 + h // 2) * 256 + qc * 128 + 128],
                        rhs=vhat_sb[:, kb * VW + h * VH: kb * VW + (h + 1) * VH],
                        start=(start and h % 2 == 0),
                        stop=(stop and h % 2 == 1), skip_group_check=True)

            def norm_mul(aug, qc):
                a4 = aug.rearrange("p (h x) -> p h x", h=HPC)  # x = 256
                recip = small.tile([128, HPC], f32, name="recip")
                nc.vector.reciprocal(
                    recip.rearrange("p (h o) -> p h o", h=HPC),
                    a4[:, :, qc * VH + 64: qc * VH + 65])
                y = ypool.tile([128, SH], bf16, name="y")
                nc.vector.tensor_mul(
                    y.rearrange("p (h d) -> p h d", h=HPC),
                    a4[:, :, qc * VH: qc * VH + 64],
                    recip.rearrange("p (h o) -> p h o", o=1)
                    .to_broadcast([128, HPC, 64]))
                return y

            USE_DMA_TRANSPOSE = False

            def transpose_unit(y, yt, qc):
                if USE_DMA_TRANSPOSE:
                    nc.sync.dma_start_transpose(
                        yt.rearrange("p (c t) -> p c t", c=2)[:, :, qc * 128:
                                                              (qc + 1) * 128],
                        y[:])
                    return
                tp = psqk.tile([128, 256], bf16, name="ps")
                for ch in (0, 1):
                    nc.tensor.transpose(
                        tp[:, ch * 128:(ch + 1) * 128],
                        y[:, ch * 128:(ch + 1) * 128], id_sb[:])
                nc.vector.tensor_copy(
                    yt.rearrange("p (c t) -> p c t", c=2)[:, :, qc * 128:
                                                          (qc + 1) * 128],
                    tp.rearrange("p (c t) -> p c t", c=2))

            def ag_unit(qq, yt):
                nc.sync.dma_start(
                    agin[qq].rearrange("(c p) t -> p c t", c=2),
                    yt.rearrange("p (c t) -> p c t", c=2))
                nc.gpsimd.collective_compute(
                    "AllGather", mybir.AluOpType.bypass,
                    replica_groups=GROUPS,
                    ins=[agin[qq].opt()], outs=[agout[qq].opt()])

            def attention_qtile(qq):
                """One 256-wide q tile over all 4 heads.  Single k-block
                sweep: QK^T/exp, both q-chunks' AV, and filler units
                interleaved so the PE keeps busy while ACT drains the exps.
                The y normalization runs on DVE right away, but the PE
                transposes + AllGather of this tile are deferred into the
                next tile's sweep (fillq front) to keep them off the PE
                critical path."""
                q0 = qq * 256
                nkb = 2 * qq + 2
                yt = ytp.tile([128, 512], bf16, name="yt")
                aug = psaug.tile([128, 1024], f32, name="aug")
                for kb in range(nkb):
                    e = qk_exp(qq, kb, q0)
                    if kb < nkb - 1:
                        av4(aug, e, kb, 0, start=(kb == 0), stop=False)
                    av4(aug, e, kb, 1, start=False, stop=(kb == nkb - 1))
                    if kb % 2 == 1:
                        pump(1)
                y0 = norm_mul(aug, 0)
                y1 = norm_mul(aug, 1)

                def finish(q=qq, a=y0, b=y1, t=yt):
                    transpose_unit(a, t, 0)
                    transpose_unit(b, t, 1)
                    ag_unit(q, t)
                if qq < nqq - 1:
                    fillq.insert(0, (qq + 2, finish))
                else:
                    finish()

            # ---- schedule ----
            for u in proj_units(0) + proj_units(1):
                u()
            ygts = {}
            for qq in range(nqq):
                if qq == 2:
                    fillq.extend((4, u) for u in proj_units(2))
                if qq == 4:
                    fillq.extend((6, u) for u in proj_units(3))
                if qq >= 2:
                    ygts[qq - 2] = yg_pull(qq - 2)
                if qq == 6:
                    for e in range(4):
                        fillq.extend((99, u) for u in o_units(e, ygts[e]))
                if qq == 7:
                    for e in (4, 5):
                        fillq.extend((99, u) for u in o_units(e, ygts[e]))
                pump_due(qq)
                attention_qtile(qq)
            pump(len(fillq))
            for e in (6, 7):
                for u in o_units(e, yg_pull(e)):
                    u()

    nc.compile()
    return nc


def _tri_np():
    ki = np.arange(128)[:, None]
    qi = np.arange(128)[None, :]
    return (qi >= ki).astype(np.float32).astype(BF16)


def _block(a, w):
    """[C, w] -> [128, NCB*w] partition-blocked bf16."""
    return np.ascontiguousarray(
        a.reshape(NCB, 128, w).transpose(1, 0, 2).reshape(128, NCB * w)).astype(BF16)


def _prep_inputs(x, Wq, Wk, Wv, Wo, t_len):
    tri = _tri_np()
    ident = np.eye(128, dtype=np.float32).astype(BF16)
    in_maps = []
    for c in range(N_CORES):
        b, hg = divmod(c, 4)
        sl = slice(hg * SH, (hg + 1) * SH)
        in_maps.append({
            "xT": _block(x[b].T, t_len),
            "wqT": _block(Wq[sl, :].T, SH),
            "wkT": _block(Wk[sl, :].T, SH),
            "wvT": _block(Wv[sl, :].T, SH),
            "woT": _block(Wo[sl, :].T, SH),
            "tri": tri,
            "ident": ident,
        })
    return in_maps


def _assemble(results, t_len):
    out = np.empty((B, t_len, C), dtype=np.float32)
    for c in range(N_CORES):
        b, hg = divmod(c, 4)
        out[b, :, hg * SH:(hg + 1) * SH] = results[c]["out"].T.astype(np.float32)
    return out


def get_nc(t_len=T):
    if t_len not in _CACHE:
        _CACHE[t_len] = _build(t_len)
    return _CACHE[t_len]


def kernel(x, Wq, Wk, Wv, Wo):
    from concourse import bass_utils
    x = np.asarray(x, dtype=np.float32)
    nc = get_nc(T)
    in_maps = _prep_inputs(x, np.asarray(Wq), np.asarray(Wk), np.asarray(Wv),
                           np.asarray(Wo), T)
    res = bass_utils.run_bass_kernel_spmd(nc, in_maps, core_ids=list(range(N_CORES)))
    return _assemble(res.results, T)


# revision 32
# speedup vs baseline: 1.7927x; 1.0794x over previous
"""Distributed causal self-attention kernel for one TRN2 chip (8 NeuronCores).

Problem: y = CausalSelfAttention(x) with B=2, T=2048, C=1024, 16 heads x 64.

Sharding (per core c = b*4 + hg;  b = batch, hg = head-group of 4 heads):
  - Q/K/V projections: column-sharded per head group (each core computes its
    4 heads' Q,K,V from the full x of its batch).
  - Attention: fully local (4 heads per core), flash-style single pass.
    Scores are kept transposed (s^T[k, q]); AV runs "transposed" too:
    y[q, d] = e^T . [V | 1]  so each AV matmul streams only 65 columns and
    the softmax denominator lands as a per-partition (per-q) column, making
    normalization a cheap per-partition multiply.
  - y chunks are PE-transposed back to y^T[c, t] and AllGathered within each
    batch group of 4 cores in 8 t-slices of 256 so comm overlaps compute.
  - o_proj: each core computes its own 256 output columns from the full
    gathered y^T -> output shards are disjoint; the host just concatenates.

All matmuls run in bf16 (fp32 accumulation in PSUM); inputs are converted to
bf16 on the host. QK^T matmuls (contraction dim 64) are packed two-per-PE
via tile_position row tiling. Exp is sized to the exact causal area
(diagonal tiles use narrowed strided APs).
"""
import sys
sys.path.insert(0, '/opt/trn_rl_repo')
import numpy as np
import ml_dtypes

B, T, C = 2, 2048, 1024
NH, HD = 16, 64
N_CORES = 8
GROUPS = [[0, 1, 2, 3], [4, 5, 6, 7]]
HPC = NH // 4             # heads per core = 4
SH = HPC * HD             # per-core projection width = 256
NCB = C // 128            # contraction blocks = 8
VH = 68                   # padded [V_h | 1 | pad] group width
VW = HPC * VH             # vhat row width per t-chunk = 272
BF16 = ml_dtypes.bfloat16

_CACHE = {}


def _build(t_len):
    import concourse.bass as bass
    import concourse.bacc as bacc
    import concourse.tile as tile
    import concourse.mybir as mybir
    dt = mybir.dt
    f32, bf16 = dt.float32, dt.bfloat16

    ntc = t_len // 128        # t chunks of 128 (16)
    nqq = t_len // 256        # q tiles of 256 == AllGather slices (8)
    ntt = t_len // 512        # projection t tiles (4)

    nc = bacc.Bacc("TRN2", target_bir_lowering=False, debug=False,
                   num_devices=N_CORES)
    # inputs arrive pre-blocked on the host: [(cblk p) ...] -> [p, cblk*...]
    xT = nc.dram_tensor("xT", [128, NCB * t_len], bf16, kind="ExternalInput")
    wq = nc.dram_tensor("wqT", [128, NCB * SH], bf16, kind="ExternalInput")
    wk = nc.dram_tensor("wkT", [128, NCB * SH], bf16, kind="ExternalInput")
    wv = nc.dram_tensor("wvT", [128, NCB * SH], bf16, kind="ExternalInput")
    wo = nc.dram_tensor("woT", [128, NCB * SH], bf16, kind="ExternalInput")
    tri = nc.dram_tensor("tri", [128, 128], bf16, kind="ExternalInput")
    ident = nc.dram_tensor("ident", [128, 128], bf16, kind="ExternalInput")
    out = nc.dram_tensor("out", [SH, t_len], bf16, kind="ExternalOutput")

    with tile.TileContext(nc) as tc:
        with tc.tile_pool(name="big", bufs=1) as big, \
             tc.tile_pool(name="epool", bufs=20) as epool, \
             tc.tile_pool(name="ypool", bufs=8) as ypool, \
             tc.tile_pool(name="ytp", bufs=6) as ytp, \
             tc.tile_pool(name="small", bufs=4) as small, \
             tc.tile_pool(name="ygp", bufs=8) as ygp, \
             tc.tile_pool(name="stp", bufs=3) as stp, \
             tc.tile_pool(name="psqk", bufs=3, space="PSUM") as psqk, \
             tc.tile_pool(name="psaug", bufs=1, space="PSUM") as psaug, \
             tc.tile_pool(name="dram", bufs=1, space="DRAM") as dram:

            # ---- resident SBUF tensors ----
            xt = big.tile([128, NCB * t_len], bf16)       # x^T, c-blocked
            wq_sb = big.tile([128, NCB * SH], bf16)
            wk_sb = big.tile([128, NCB * SH], bf16)
            wv_sb = big.tile([128, NCB * SH], bf16)
            wo_sb = big.tile([128, NCB * SH], bf16)
            tri_sb = big.tile([128, 128], bf16)
            id_sb = big.tile([128, 128], bf16)
            qt_sb = big.tile([128, 2 * t_len], bf16)      # Q^T, pair-blocked
            kt_sb = big.tile([128, 2 * t_len], bf16)
            vhat_sb = big.tile([128, ntc * VW], bf16)     # [V_h | 1] per head

            # Coalesced input DMAs, first-consumer order.  x arrives one
            # 512-wide t tile at a time (all 8 c-blocks, strided).
            def x_tile_dma(ti):
                nc.sync.dma_start(
                    xt.rearrange("p (k t) -> p k t", k=NCB)
                    [:, :, ti * 512:(ti + 1) * 512],
                    xT.rearrange("p (k t) -> p k t", k=NCB)
                    [:, :, ti * 512:(ti + 1) * 512])

            nc.sync.dma_start(wq_sb[:, 0:SH], wq[:, 0:SH])
            for k in range(0, NCB, 2):
                nc.sync.dma_start(
                    xt.rearrange("p (k t) -> p k t", k=NCB)[:, k:k + 2, 0:512],
                    xT.rearrange("p (k t) -> p k t", k=NCB)[:, k:k + 2, 0:512])
            nc.sync.dma_start(wq_sb[:, SH:], wq[:, SH:])
            nc.sync.dma_start(wk_sb[:], wk[:])
            nc.sync.dma_start(wv_sb[:], wv[:])
            nc.sync.dma_start(tri_sb[:], tri[:])
            nc.sync.dma_start(id_sb[:], ident[:])
            for ti in range(1, ntt):
                x_tile_dma(ti)
            nc.sync.dma_start(wo_sb[:], wo[:])
            nc.gpsimd.memset(vhat_sb[:], 1.0)

            # ---- DRAM bounce buffers for the AllGathers (one per q-tile) --
            agin = [dram.tile([2 * 128, 256], bf16, name=f"agin{qq}")
                    for qq in range(nqq)]
            agout = [dram.tile([NCB * 128, 256], bf16, name=f"agout{qq}")
                     for qq in range(nqq)]

            def proj_unit(w_sb, dst, pair, t0):
                """One q/k projection psum group: [d128, t512]."""
                ps = psqk.tile([128, 512], f32, name="ps")
                for k in range(NCB):
                    nc.tensor.matmul(
                        ps[:],
                        lhsT=w_sb[:, k * SH + pair * 128:
                                  k * SH + (pair + 1) * 128],
                        rhs=xt[:, k * t_len + t0: k * t_len + t0 + 512],
                        start=(k == 0), stop=(k == NCB - 1))
                nc.vector.tensor_copy(
                    dst[:, pair * t_len + t0: pair * t_len + t0 + 512], ps[:])

            def v_unit(tch):
                """One V projection psum group: [t128, 4h x 64d] -> vhat."""
                ps = psqk.tile([128, SH], f32, name="ps")
                for k in range(NCB):
                    nc.tensor.matmul(
                        ps[:],
                        lhsT=xt[:, k * t_len + tch * 128:
                                k * t_len + (tch + 1) * 128],
                        rhs=wv_sb[:, k * SH:(k + 1) * SH],
                        start=(k == 0), stop=(k == NCB - 1))
                nc.vector.tensor_copy(
                    vhat_sb.rearrange("p (c h d) -> p c h d",
                                      c=ntc, h=HPC)[:, tch, :, 0:64],
                    ps.rearrange("p (h d) -> p h d", h=HPC))

            def proj_units(ti):
                t0 = ti * 512
                return ([(lambda p=pair, w=w_sb, d=dst: proj_unit(w, d, p, t0))
                         for w_sb, dst in ((wq_sb, qt_sb), (wk_sb, kt_sb))
                         for pair in (0, 1)] +
                        [(lambda t=tch: v_unit(t))
                         for tch in range(ti * 4, ti * 4 + 4)])

            def yg_pull(qq):
                t = ygp.tile([128, NCB * 256], bf16, name="yg")
                nc.sync.dma_start(
                    t.rearrange("p (r t) -> p r t", r=NCB),
                    agout[qq].rearrange("(r p) t -> p r t", r=NCB))
                return t

            def o_unit(qq, ygt, st2, m):
                """One o_proj output block [o128, t256] for t slice qq."""
                ps = psqk.tile([128, 256], f32, name="ps")
                for cb in range(NCB):
                    nc.tensor.matmul(
                        ps[:],
                        lhsT=wo_sb[:, cb * SH + m * 128: cb * SH + (m + 1) * 128],
                        rhs=ygt[:, cb * 256:(cb + 1) * 256],
                        start=(cb == 0), stop=(cb == NCB - 1))
                nc.vector.tensor_copy(st2[:, m * 256:(m + 1) * 256], ps[:])
                if m == 1:
                    nc.sync.dma_start(
                        out[:, qq * 256:(qq + 1) * 256]
                        .rearrange("(m p) t -> p m t", m=2),
                        st2.rearrange("p (m t) -> p m t", m=2))

            def o_units(qq, ygt):
                st2 = stp.tile([128, 512], bf16, name="st")
                return [(lambda m=m, y=ygt, s=st2: o_unit(qq, y, s, m))
                        for m in (0, 1)]

            # ---- filler: PE work units woven into ACT-bound QK stretches.
            # Entries are (deadline_qq, fn): the unit must have run before
            # attention_qtile(deadline_qq) starts (projections feeding it).
            fillq = []
            pendfin = []

            def pump(n=1):
                for _ in range(min(n, len(fillq))):
                    fillq.pop(0)[1]()

            def pump_due(qq):
                while any(d <= qq for d, _ in fillq):
                    pump(1)

            def qk_exp(qq, kb, q0):
                """QK^T + exp for one k-block; returns e tile.  PSUM column
                groups are ordered (h01, pair) so each 2KB bank only receives
                matmuls from a single PE row-tile position (bank0 <- rows
                0-63, bank1 <- rows 64-127) -- mixing positions in one bank
                does not compile.  On-diagonal blocks get the causal tri mask
                applied on DVE after the exp."""
                j = kb - 2 * qq
                qa = 128 if j == 1 else 0
                qk = psqk.tile([128, 1024], f32, name="ps")
                for pair in (0, 1):
                    for h01 in (0, 1):
                        g = h01 * 2 + pair
                        nc.tensor.matmul(
                            qk[:, g * 256 + qa: (g + 1) * 256],
                            lhsT=kt_sb[h01 * 64:(h01 + 1) * 64,
                                       pair * t_len + kb * 128:
                                       pair * t_len + (kb + 1) * 128],
                            rhs=qt_sb[h01 * 64:(h01 + 1) * 64,
                                      pair * t_len + q0 + qa:
                                      pair * t_len + q0 + 256],
                            start=True, stop=True,
                            tile_position=(h01 * 64, 0))
                e = epool.tile([128, 1024], bf16, name="e")
                if qa == 0:
                    nc.scalar.activation(e[:], qk[:],
                                         mybir.ActivationFunctionType.Exp,
                                         scale=1.0 / np.sqrt(HD))
                else:
                    e4 = e.rearrange("p (g q) -> p g q", g=HPC)
                    qk4 = qk.rearrange("p (g q) -> p g q", g=HPC)
                    nc.scalar.activation(e4[:, :, qa:256], qk4[:, :, qa:256],
                                         mybir.ActivationFunctionType.Exp,
                                         scale=1.0 / np.sqrt(HD))
                if j >= 0:
                    e4 = e.rearrange("p (g q) -> p g q", g=HPC)
                    nc.vector.tensor_mul(
                        e4[:, :, qa:qa + 128], e4[:, :, qa:qa + 128],
                        tri_sb[:].rearrange("p (o q) -> p o q", o=1)
                        .to_broadcast([128, HPC, 128]))
                return e

            def av4(aug, etile, kb, qc, start, stop):
                """AV for one (k-block, q-chunk): per head, y[q,d]+rowsum.
                qc0 lives at cols h*512+0:65 of aug, qc1 at h*512+65:130 --
                same PSUM bank, one accumulation region per head: only the
                very first write per bank carries start=True, only the very
                last carries stop=True (pending-zero covers qc1's first
                write)."""
                for h in range(HPC):
                    nc.tensor.matmul(
                        aug[:, h * 256 + qc * VH: h * 256 + qc * VH + VH],
                        lhsT=etile[:, ((h % 2) * 2 + h // 2) * 256 + qc * 128:
                                   ((h % 2) * 2 + h // 2) * 256 + qc * 128 + 128],
                        rhs=vhat_sb[:, kb * VW + h * VH: kb * VW + (h + 1) * VH],
                        start=(start and h % 2 == 0),
                        stop=(stop and h % 2 == 1), skip_group_check=True)

            def norm_mul(aug, qc):
                a4 = aug.rearrange("p (h x) -> p h x", h=HPC)  # x = 256
                recip = small.tile([128, HPC], f32, name="recip")
                nc.vector.reciprocal(
                    recip.rearrange("p (h o) -> p h o", h=HPC),
                    a4[:, :, qc * VH + 64: qc * VH + 65])
                y = ypool.tile([128, SH], bf16, name="y")
                nc.vector.tensor_mul(
                    y.rearrange("p (h d) -> p h d", h=HPC),
                    a4[:, :, qc * VH: qc * VH + 64],
                    recip.rearrange("p (h o) -> p h o", o=1)
                    .to_broadcast([128, HPC, 64]))
                return y

            USE_DMA_TRANSPOSE = False

            def transpose_unit(y, yt, qc):
                if USE_DMA_TRANSPOSE:
                    nc.sync.dma_start_transpose(
                        yt.rearrange("p (c t) -> p c t", c=2)[:, :, qc * 128:
                                                              (qc + 1) * 128],
                        y[:])
                    return
                tp = psqk.tile([128, 256], bf16, name="ps")
                for ch in (0, 1):
                    nc.tensor.transpose(
                        tp[:, ch * 128:(ch + 1) * 128],
                        y[:, ch * 128:(ch + 1) * 128], id_sb[:])
                nc.vector.tensor_copy(
                    yt.rearrange("p (c t) -> p c t", c=2)[:, :, qc * 128:
                                                          (qc + 1) * 128],
                    tp.rearrange("p (c t) -> p c t", c=2))

            def ag_unit(qq, yt):
                nc.sync.dma_start(
                    agin[qq].rearrange("(c p) t -> p c t", c=2),
                    yt.rearrange("p (c t) -> p c t", c=2))
                nc.gpsimd.collective_compute(
                    "AllGather", mybir.AluOpType.bypass,
                    replica_groups=GROUPS,
                    ins=[agin[qq].opt()], outs=[agout[qq].opt()])

            def attention_qtile(qq):
                """One 256-wide q tile over all 4 heads.  Single k-block
                sweep: QK^T/exp, both q-chunks' AV, and filler units
                interleaved so the PE keeps busy while ACT drains the exps.
                The y normalization runs on DVE right away, but the PE
                transposes + AllGather of this tile are deferred into the
                next tile's sweep (fillq front) to keep them off the PE
                critical path."""
                q0 = qq * 256
                nkb = 2 * qq + 2
                yt = ytp.tile([128, 512], bf16, name="yt")
                aug = psaug.tile([128, 1024], f32, name="aug")
                for kb in range(nkb):
                    e = qk_exp(qq, kb, q0)
                    if kb < nkb - 1:
                        av4(aug, e, kb, 0, start=(kb == 0), stop=False)
                    av4(aug, e, kb, 1, start=False, stop=(kb == nkb - 1))
                    if kb == 5 and pendfin:
                        pendfin.pop(0)()
                    elif kb % 2 == 1:
                        pump(1)
                y0 = norm_mul(aug, 0)
                y1 = norm_mul(aug, 1)

                def finish(q=qq, a=y0, b=y1, t=yt):
                    transpose_unit(a, t, 0)
                    transpose_unit(b, t, 1)
                    ag_unit(q, t)
                if qq < nqq - 1:
                    pendfin.append(finish)
                else:
                    while pendfin:
                        pendfin.pop(0)()
                    finish()

            # ---- schedule ----
            for u in proj_units(0) + proj_units(1):
                u()
            ygts = {}
            for qq in range(nqq):
                if qq == 2:
                    fillq.extend((4, u) for u in proj_units(2))
                if qq == 4:
                    fillq.extend((6, u) for u in proj_units(3))
                if qq >= 2:
                    ygts[qq - 2] = yg_pull(qq - 2)
                if qq == 6:
                    for e in range(4):
                        fillq.extend((99, u) for u in o_units(e, ygts[e]))
                if qq == 7:
                    for e in (4, 5):
                        fillq.extend((99, u) for u in o_units(e, ygts[e]))
                pump_due(qq)
                attention_qtile(qq)
            pump(len(fillq))
            for e in (6, 7):
                for u in o_units(e, yg_pull(e)):
                    u()

    nc.compile()
    return nc


def _tri_np():
    ki = np.arange(128)[:, None]
    qi = np.arange(128)[None, :]
    return (qi >= ki).astype(np.float32).astype(BF16)


def _block(a, w):
    """[C, w] -> [128, NCB*w] partition-blocked bf16."""
    return np.ascontiguousarray(
        a.reshape(NCB, 128, w).transpose(1, 0, 2).reshape(128, NCB * w)).astype(BF16)


def _prep_inputs(x, Wq, Wk, Wv, Wo, t_len):
    tri = _tri_np()
    ident = np.eye(128, dtype=np.float32).astype(BF16)
    in_maps = []
    for c in range(N_CORES):
        b, hg = divmod(c, 4)
        sl = slice(hg * SH, (hg + 1) * SH)
        in_maps.append({
            "xT": _block(x[b].T, t_len),
            "wqT": _block(Wq[sl, :].T, SH),
            "wkT": _block(Wk[sl, :].T, SH),
            "wvT": _block(Wv[sl, :].T, SH),
            "woT": _block(Wo[sl, :].T, SH),
            "tri": tri,
            "ident": ident,
        })
    return in_maps


def _assemble(results, t_len):
    out = np.empty((B, t_len, C), dtype=np.float32)
    for c in range(N_CORES):
        b, hg = divmod(c, 4)
        out[b, :, hg * SH:(hg + 1) * SH] = results[c]["out"].T.astype(np.float32)
    return out


def get_nc(t_len=T):
    if t_len not in _CACHE:
        _CACHE[t_len] = _build(t_len)
    return _CACHE[t_len]


def kernel(x, Wq, Wk, Wv, Wo):
    from concourse import bass_utils
    x = np.asarray(x, dtype=np.float32)
    nc = get_nc(T)
    in_maps = _prep_inputs(x, np.asarray(Wq), np.asarray(Wk), np.asarray(Wv),
                           np.asarray(Wo), T)
    res = bass_utils.run_bass_kernel_spmd(nc, in_maps, core_ids=list(range(N_CORES)))
    return _assemble(res.results, T)
